# revision 17
# baseline (speedup 1.0000x reference)
"""NCC loss (9x9x9 box normalized cross-correlation) on 8 TRN2 NeuronCores.

Inputs: y_pred, y_true f32 (2,1,128,128,128). Output: scalar f32 loss.

Sharding: D axis (dim 2) split 4-ways per batch -> 8 slabs of 32 D-slices,
each with a 4-slice halo (host zero-pads volume edges). Inputs are converted
to bf16 on the host (same precision as the on-device copy the previous
version did) and packed into a [104, 64, 128] layout: partitions 0..39 hold
the 40 halo'd d-rows for h-block 0 (h 0..63), partitions 64..103 hold them
for h-block 1 (h 64..127), rows 40..63 / 104..127 are zero.

Per core, separable box filter as three matmul passes (contract D, then W,
then H) so every intermediate is a full-128-partition tile:

  prep  : I*I, J*J, I*J products in bf16                  (DVE/ACT)
  P_D   : per h, lhsT=vol[d,w] slab, rhs=banded BD[40,32] -> t1 [w,(h,d')]
  P_W   : per d', lhsT=t1[w,h], rhs=band BW[128,128]      -> t2 [h,(d',w')]
  P_H   : stationary band BH, rhs=t2 chunks of 512        -> PSUM [h',512]
  ptw   : cc = cross^2/(I_var*J_var) with the three big PSUM subtractions
          done ON THE PE via accumulating -identity matmuls, reciprocal via
          the fast bit-trick custom DVE op, final mean via ones-matmul
          reduction accumulated in PSUM.
Host: sum per-core [128,8] partials, loss = -sum / N.
"""

import math

import numpy as np
import ml_dtypes

import concourse.bacc as bacc
import concourse.tile as tile
from concourse import mybir
from concourse.bass_utils import run_bass_kernel_spmd

F32 = mybir.dt.float32
BF16 = mybir.dt.bfloat16
ALU = mybir.AluOpType
ACTF = mybir.ActivationFunctionType

B, D, H, W = 2, 128, 128, 128
DL, PAD = 32, 4
DH = DL + 2 * PAD            # 40
SQS = math.sqrt(1.0 / 729.0)
N_TOT = float(B * D * H * W)

_CACHE = {}


def _build():
    nc = bacc.Bacc(trn_type="TRN2", target_bir_lowering=False)

    i_dram = nc.dram_tensor("i_pk", [104, 64, 128], BF16, kind="ExternalInput")
    j_dram = nc.dram_tensor("j_pk", [104, 64, 128], BF16, kind="ExternalInput")
    out_dram = nc.dram_tensor("partials", [128, 8], F32, kind="ExternalOutput")

    with tile.TileContext(nc) as tc:
        with (
            tc.tile_pool(name="bands", bufs=1) as bands,
            tc.tile_pool(name="stage", bufs=1) as stage,
            tc.tile_pool(name="accp", bufs=1) as accp,
        ):
            # ---------- band / constant matrices ----------
            # BD[p, j] = 1 iff j <= p <= j+8, duplicated at partition 64.
            bd = bands.tile([104, 32], BF16)
            nc.gpsimd.memset(bd[0:40, :], 1.0)
            nc.gpsimd.affine_select(bd[0:40, :], bd[0:40, :], pattern=[[-1, 32]],
                                    compare_op=ALU.is_ge, fill=0.0,
                                    base=0, channel_multiplier=1)
            nc.gpsimd.affine_select(bd[0:40, :], bd[0:40, :], pattern=[[1, 32]],
                                    compare_op=ALU.is_ge, fill=0.0,
                                    base=8, channel_multiplier=-1)
            nc.sync.dma_start(out=bd[64:104, :], in_=bd[0:40, :])

            # BW = BH: [p, j] = 1 iff |p - j| <= 4
            bw = bands.tile([128, 128], BF16)
            nc.gpsimd.memset(bw[:, :], 1.0)
            nc.gpsimd.affine_select(bw[:, :], bw[:, :], pattern=[[-1, 128]],
                                    compare_op=ALU.is_ge, fill=0.0,
                                    base=PAD, channel_multiplier=1)
            nc.gpsimd.affine_select(bw[:, :], bw[:, :], pattern=[[1, 128]],
                                    compare_op=ALU.is_ge, fill=0.0,
                                    base=PAD, channel_multiplier=-1)

            # -identity for PE-side subtraction
            negI = bands.tile([128, 128], BF16)
            nc.gpsimd.memset(negI[:, :], -1.0)
            nc.gpsimd.affine_select(negI[:, :], negI[:, :], pattern=[[-1, 128]],
                                    compare_op=ALU.is_ge, fill=0.0,
                                    base=0, channel_multiplier=1)
            nc.gpsimd.affine_select(negI[:, :], negI[:, :], pattern=[[1, 128]],
                                    compare_op=ALU.is_ge, fill=0.0,
                                    base=0, channel_multiplier=-1)

            ones = bands.tile([128, 1], BF16)
            nc.gpsimd.memset(ones[:, :], 1.0)

            # t2 tiles live until the end
            t2 = [stage.tile([128, 32, 128], BF16, name=f"t2_{v}")
                  for v in range(5)]

            # ---------- inputs + products ----------
            # pool stack (LIFO): t1 -> psD -> vols; vols popped after P_D.
            t1p = tc.tile_pool(name="t1", bufs=1)
            t1pool = t1p.__enter__()
            t1 = [t1pool.tile([128, 128, 32], BF16, name=f"t1_{v}")
                  for v in range(5)]
            psD = tc.tile_pool(name="psD", bufs=2, space="PSUM")
            psDp = psD.__enter__()
            volp = tc.tile_pool(name="vols", bufs=1)
            vols = volp.__enter__()
            vi = vols.tile([104, 64, 128], BF16, name="vi")
            vj = vols.tile([104, 64, 128], BF16, name="vj")
            vi2 = vols.tile([104, 64, 128], BF16, name="vi2")
            vj2 = vols.tile([104, 64, 128], BF16, name="vj2")
            vij = vols.tile([104, 64, 128], BF16, name="vij")
            for q in range(4):
                s = slice(q * 16, q * 16 + 16)
                nc.sync.dma_start(out=vi[:, s, :], in_=i_dram[:, s, :])
            for q in range(4):
                s = slice(q * 16, q * 16 + 16)
                nc.sync.dma_start(out=vj[:, s, :], in_=j_dram[:, s, :])
                nc.vector.tensor_tensor(out=vi2[:, s, :], in0=vi[:, s, :],
                                        in1=vi[:, s, :], op=ALU.mult)
            for q in range(4):
                s = slice(q * 16, q * 16 + 16)
                nc.scalar.square(vj2[:, s, :], vj[:, s, :])
                nc.vector.tensor_tensor(out=vij[:, s, :], in0=vi[:, s, :],
                                        in1=vj[:, s, :], op=ALU.mult)

            VOLS = [vi, vi2, vj, vj2, vij]

            # ---------- P_D: contract D -> t1 [w, (h 128, d' 32)] ----------
            # Pool cannot touch PSUM: evacuations alternate ACT / DVE only.
            nev = 0
            for v in range(5):
                vol = VOLS[v]
                for hb in range(2):           # 4-bank tiles of 64 h
                    ps = psDp.tile([128, 64, 32], F32, tag="psD")
                    for k in range(64):
                        h = hb * 64 + k
                        b, hl = h >> 6, h & 63
                        nc.tensor.matmul(
                            out=ps[:, k, :],
                            lhsT=vol[64 * b:64 * b + 40, hl, :],
                            rhs=bd[64 * b:64 * b + 40, :])
                    dst = t1[v][:, hb * 64:hb * 64 + 64, :]
                    if nev % 2 == 0:
                        nc.scalar.copy(dst, ps[:, :, :])
                    else:
                        nc.vector.tensor_copy(dst, ps[:, :, :])
                    nev += 1

            volp.__exit__(None, None, None)
            psD.__exit__(None, None, None)

            # ---------- P_W + P_H + pointwise, pipelined per d'-block ----
            # P_W ordered d'-block-outer so chunk c's P_H + pointwise can
            # trail one block behind P_W(c+1): Pool's pointwise overlaps
            # ACT/DVE evacuations, P_W matmuls fill PE between P_H chunks.
            psW = tc.tile_pool(name="psW", bufs=1, space="PSUM")
            psWp = psW.__enter__()
            psH = tc.tile_pool(name="psH", bufs=6, space="PSUM")
            psHp = psH.__enter__()
            psR = tc.tile_pool(name="psR", bufs=1, space="PSUM")
            psRp = psR.__enter__()
            ptwp = tc.tile_pool(name="ptw", bufs=3)
            ptw = ptwp.__enter__()

            acc_ps = psRp.tile([128, 8], F32)
            pend = {}

            def emit_chunk(c):
                # VOLS/t2 order is [I, I2, J, J2, IJ]
                rhs = [t2[v][:, c * 4:c * 4 + 4, :].rearrange(
                    "p a b -> p (a b)") for v in range(5)]
                psI = psHp.tile([128, 512], F32, tag="psH")
                psJ = psHp.tile([128, 512], F32, tag="psH")
                nc.tensor.matmul(out=psI[:, :], lhsT=bw[:, :], rhs=rhs[0])
                nc.tensor.matmul(out=psJ[:, :], lhsT=bw[:, :], rhs=rhs[2])
                psI2 = psHp.tile([128, 512], F32, tag="psH")
                psJ2 = psHp.tile([128, 512], F32, tag="psH")
                psIJ = psHp.tile([128, 512], F32, tag="psH")
                nc.tensor.matmul(out=psI2[:, :], lhsT=bw[:, :], rhs=rhs[1],
                                 start=True, stop=False)
                nc.tensor.matmul(out=psJ2[:, :], lhsT=bw[:, :], rhs=rhs[3],
                                 start=True, stop=False)
                nc.tensor.matmul(out=psIJ[:, :], lhsT=bw[:, :], rhs=rhs[4],
                                 start=True, stop=False)

                ap = ptw.tile([128, 512], BF16, tag="ap", name="ap")
                bp = ptw.tile([128, 512], BF16, tag="bp", name="bp")
                nc.scalar.mul(ap[:, :], psI[:, :], SQS)
                nc.scalar.mul(bp[:, :], psJ[:, :], SQS)

                qI = ptw.tile([128, 512], BF16, tag="qI", name="qI")
                qJ = ptw.tile([128, 512], BF16, tag="qJ", name="qJ")
                m = ptw.tile([128, 512], BF16, tag="m", name="m")
                nc.gpsimd.tensor_tensor(out=qI[:, :], in0=ap[:, :],
                                        in1=ap[:, :], op=ALU.mult)
                nc.gpsimd.tensor_tensor(out=qJ[:, :], in0=bp[:, :],
                                        in1=bp[:, :], op=ALU.mult)
                nc.vector.tensor_tensor(out=m[:, :], in0=ap[:, :],
                                        in1=bp[:, :], op=ALU.mult)
                pend[c] = (psI2, psJ2, psIJ, qI, qJ, m)

            def subs_chunk(c):
                psI2, psJ2, psIJ, qI, qJ, m = pend[c]
                nc.tensor.matmul(out=psI2[:, :], lhsT=negI[:, :], rhs=qI[:, :],
                                 start=False, stop=True)
                nc.tensor.matmul(out=psJ2[:, :], lhsT=negI[:, :], rhs=qJ[:, :],
                                 start=False, stop=True)
                nc.tensor.matmul(out=psIJ[:, :], lhsT=negI[:, :], rhs=m[:, :],
                                 start=False, stop=True)

            def finish_chunk(c):
                psI2, psJ2, psIJ, qI, qJ, m = pend.pop(c)
                ivp = ptw.tile([128, 512], BF16, tag="ivp", name="ivp")
                crp = ptw.tile([128, 512], BF16, tag="crp", name="crp")
                nc.scalar.copy(ivp[:, :], psI2[:, :])
                nc.scalar.copy(crp[:, :], psIJ[:, :])

                den = ptw.tile([128, 512], F32, tag="den", name="den")
                nc.vector.tensor_tensor(out=den[:, :], in0=psJ2[:, :],
                                        in1=ivp[:, :], op=ALU.mult)
                rec = ptw.tile([128, 512], F32, tag="rec", name="rec")
                nc.vector.reciprocal_approx_fast(out=rec[:, :], in_=den[:, :])

                t = ptw.tile([128, 512], BF16, tag="t", name="t")
                nc.vector.tensor_tensor(out=t[:, :], in0=crp[:, :],
                                        in1=rec[:, :], op=ALU.mult)
                cc = ptw.tile([128, 512], BF16, tag="cc", name="cc")
                nc.gpsimd.tensor_tensor(out=cc[:, :], in0=t[:, :],
                                        in1=crp[:, :], op=ALU.mult)
                return cc

            def reduce_chunk(c, cc):
                for k in range(4):
                    nc.tensor.matmul(out=acc_ps[:, c:c + 1],
                                     lhsT=cc[:, 128 * k:128 * k + 128],
                                     rhs=ones[:, :],
                                     start=(k == 0), stop=(k == 3))

            ccs = {}
            for db in range(11):
                if db < 8:
                    for v in range(5):
                        ps = psWp.tile([128, 4, 128], F32, tag="psW")
                        for k in range(4):
                            dp = db * 4 + k
                            nc.tensor.matmul(out=ps[:, k, :],
                                             lhsT=t1[v][:, :, dp],
                                             rhs=bw[:, :])
                        dst = t2[v][:, db * 4:db * 4 + 4, :]
                        if nev % 2 == 0:
                            nc.scalar.copy(dst, ps[:, :, :])
                        else:
                            nc.vector.tensor_copy(dst, ps[:, :, :])
                        nev += 1
                if 2 <= db <= 9:
                    subs_chunk(db - 2)
                if 1 <= db <= 8:
                    emit_chunk(db - 1)
                if 2 <= db <= 9:
                    ccs[db - 2] = finish_chunk(db - 2)
                if db >= 3:
                    reduce_chunk(db - 3, ccs.pop(db - 3))

            accs = accp.tile([128, 8], F32)
            nc.scalar.copy(accs[:, :], acc_ps[:, :])
            nc.sync.dma_start(out=out_dram[:, :], in_=accs[:, :])
            ptwp.__exit__(None, None, None)
            psR.__exit__(None, None, None)
            psH.__exit__(None, None, None)
            psW.__exit__(None, None, None)
            t1p.__exit__(None, None, None)

    nc.compile()
    return nc


def kernel(y_pred: np.ndarray, y_true: np.ndarray) -> np.ndarray:
    y_pred = np.asarray(y_pred, dtype=np.float32)
    y_true = np.asarray(y_true, dtype=np.float32)

    if "nc" not in _CACHE:
        _CACHE["nc"] = _build()
    nc = _CACHE["nc"]

    ib = y_true.astype(ml_dtypes.bfloat16)
    jb = y_pred.astype(ml_dtypes.bfloat16)

    in_maps = []
    for core in range(8):
        b = core // 4
        d0 = (core % 4) * DL
        lo, hi = d0 - PAD, d0 + DL + PAD
        slo, shi = max(lo, 0), min(hi, D)
        ipk = np.zeros((104, 64, 128), ml_dtypes.bfloat16)
        jpk = np.zeros((104, 64, 128), ml_dtypes.bfloat16)
        for hb in range(2):
            hs = slice(hb * 64, hb * 64 + 64)
            p0 = 64 * hb
            ipk[p0 + slo - lo:p0 + shi - lo] = ib[b, 0, slo:shi, hs, :]
            jpk[p0 + slo - lo:p0 + shi - lo] = jb[b, 0, slo:shi, hs, :]
        in_maps.append({"i_pk": ipk, "j_pk": jpk})

    res = run_bass_kernel_spmd(nc, in_maps, core_ids=list(range(8)))
    total = 0.0
    for r in res.results:
        total += float(np.asarray(r["partials"], np.float64).sum())
    return np.float32(-total / N_TOT)


if __name__ == "__main__":
    rng = np.random.default_rng(0)
    yp = rng.standard_normal((B, 1, D, H, W), dtype=np.float32)
    yt = rng.standard_normal((B, 1, D, H, W), dtype=np.float32)
    print("loss:", kernel(yp, yt))


# revision 19
# speedup vs baseline: 1.1499x; 1.1499x over previous
"""NCC loss (9x9x9 box normalized cross-correlation) on 8 TRN2 NeuronCores.

Inputs: y_pred, y_true f32 (2,1,128,128,128). Output: scalar f32 loss.

Sharding: D axis (dim 2) split 4-ways per batch -> 8 slabs of 32 D-slices,
each with a 4-slice halo (host zero-pads volume edges). Inputs are converted
to bf16 on the host (same precision as the on-device copy the previous
version did) and packed into a [104, 64, 128] layout: partitions 0..39 hold
the 40 halo'd d-rows for h-block 0 (h 0..63), partitions 64..103 hold them
for h-block 1 (h 64..127), rows 40..63 / 104..127 are zero.

Per core, separable box filter as three matmul passes (contract D, then W,
then H) so every intermediate is a full-128-partition tile:

  prep  : I*I, J*J, I*J products in bf16                  (DVE/ACT)
  P_D   : per h, lhsT=vol[d,w] slab, rhs=banded BD[40,32] -> t1 [w,(h,d')]
  P_W   : per d', lhsT=t1[w,h], rhs=band BW[128,128]      -> t2 [h,(d',w')]
  P_H   : stationary band BH, rhs=t2 chunks of 512        -> PSUM [h',512]
  ptw   : cc = cross^2/(I_var*J_var) with the three big PSUM subtractions
          done ON THE PE via accumulating -identity matmuls, reciprocal via
          the fast bit-trick custom DVE op, final mean via ones-matmul
          reduction accumulated in PSUM.
Host: sum per-core [128,8] partials, loss = -sum / N.
"""

import math

import numpy as np
import ml_dtypes

import concourse.bacc as bacc
import concourse.tile as tile
from concourse import mybir
from concourse.bass_utils import run_bass_kernel_spmd

F32 = mybir.dt.float32
BF16 = mybir.dt.bfloat16
ALU = mybir.AluOpType
ACTF = mybir.ActivationFunctionType

B, D, H, W = 2, 128, 128, 128
DL, PAD = 32, 4
DH = DL + 2 * PAD            # 40
SQS = math.sqrt(1.0 / 729.0)
N_TOT = float(B * D * H * W)

_CACHE = {}


def _build():
    nc = bacc.Bacc(trn_type="TRN2", target_bir_lowering=False)

    i_dram = nc.dram_tensor("i_pk", [104, 64, 128], BF16, kind="ExternalInput")
    j_dram = nc.dram_tensor("j_pk", [104, 64, 128], BF16, kind="ExternalInput")
    out_dram = nc.dram_tensor("partials", [128, 8], F32, kind="ExternalOutput")

    with tile.TileContext(nc) as tc:
        with (
            tc.tile_pool(name="bands", bufs=1) as bands,
            tc.tile_pool(name="stage", bufs=1) as stage,
            tc.tile_pool(name="accp", bufs=1) as accp,
        ):
            # ---------- band / constant matrices ----------
            # BD[p, j] = 1 iff j <= p <= j+8, duplicated at partition 64.
            bd = bands.tile([104, 32], BF16)
            nc.gpsimd.memset(bd[0:40, :], 1.0)
            nc.gpsimd.affine_select(bd[0:40, :], bd[0:40, :], pattern=[[-1, 32]],
                                    compare_op=ALU.is_ge, fill=0.0,
                                    base=0, channel_multiplier=1)
            nc.gpsimd.affine_select(bd[0:40, :], bd[0:40, :], pattern=[[1, 32]],
                                    compare_op=ALU.is_ge, fill=0.0,
                                    base=8, channel_multiplier=-1)
            nc.sync.dma_start(out=bd[64:104, :], in_=bd[0:40, :])

            # BW = BH: [p, j] = 1 iff |p - j| <= 4
            bw = bands.tile([128, 128], BF16)
            nc.gpsimd.memset(bw[:, :], 1.0)
            nc.gpsimd.affine_select(bw[:, :], bw[:, :], pattern=[[-1, 128]],
                                    compare_op=ALU.is_ge, fill=0.0,
                                    base=PAD, channel_multiplier=1)
            nc.gpsimd.affine_select(bw[:, :], bw[:, :], pattern=[[1, 128]],
                                    compare_op=ALU.is_ge, fill=0.0,
                                    base=PAD, channel_multiplier=-1)

            # -identity for PE-side subtraction
            negI = bands.tile([128, 128], BF16)
            nc.gpsimd.memset(negI[:, :], -1.0)
            nc.gpsimd.affine_select(negI[:, :], negI[:, :], pattern=[[-1, 128]],
                                    compare_op=ALU.is_ge, fill=0.0,
                                    base=0, channel_multiplier=1)
            nc.gpsimd.affine_select(negI[:, :], negI[:, :], pattern=[[1, 128]],
                                    compare_op=ALU.is_ge, fill=0.0,
                                    base=0, channel_multiplier=-1)

            ones = bands.tile([128, 1], BF16)
            nc.gpsimd.memset(ones[:, :], 1.0)

            # t2 tiles live until the end
            t2 = [stage.tile([128, 32, 128], BF16, name=f"t2_{v}")
                  for v in range(5)]

            # ---------- inputs + products ----------
            # pool stack (LIFO): t1 -> psD -> vols; vols popped after P_D.
            t1p = tc.tile_pool(name="t1", bufs=1)
            t1pool = t1p.__enter__()
            t1 = [t1pool.tile([128, 128, 32], BF16, name=f"t1_{v}")
                  for v in range(5)]
            psD = tc.tile_pool(name="psD", bufs=2, space="PSUM")
            psDp = psD.__enter__()
            volp = tc.tile_pool(name="vols", bufs=1)
            vols = volp.__enter__()
            vi = vols.tile([104, 64, 128], BF16, name="vi")
            vj = vols.tile([104, 64, 128], BF16, name="vj")
            vi2 = vols.tile([104, 64, 128], BF16, name="vi2")
            vj2 = vols.tile([104, 64, 128], BF16, name="vj2")
            vij = vols.tile([104, 64, 128], BF16, name="vij")
            for q in range(4):
                s = slice(q * 16, q * 16 + 16)
                nc.sync.dma_start(out=vi[:, s, :], in_=i_dram[:, s, :])
                nc.sync.dma_start(out=vj[:, s, :], in_=j_dram[:, s, :])
            for q in range(4):
                s = slice(q * 16, q * 16 + 16)
                nc.vector.tensor_tensor(out=vi2[:, s, :], in0=vi[:, s, :],
                                        in1=vi[:, s, :], op=ALU.mult)
                nc.scalar.square(vj2[:, s, :], vj[:, s, :])
                nc.vector.tensor_tensor(out=vij[:, s, :], in0=vi[:, s, :],
                                        in1=vj[:, s, :], op=ALU.mult)

            VOLS = [vi, vj, vi2, vj2, vij]

            # ---------- P_D: contract D -> t1 [w, (h 128, d' 32)] ----------
            # Pool cannot touch PSUM: evacuations alternate ACT / DVE only.
            nev = 0
            for v in range(5):
                vol = VOLS[v]
                for hb in range(2):           # 4-bank tiles of 64 h
                    ps = psDp.tile([128, 64, 32], F32, tag="psD")
                    for k in range(64):
                        h = hb * 64 + k
                        b, hl = h >> 6, h & 63
                        nc.tensor.matmul(
                            out=ps[:, k, :],
                            lhsT=vol[64 * b:64 * b + 40, hl, :],
                            rhs=bd[64 * b:64 * b + 40, :])
                    dst = t1[v][:, hb * 64:hb * 64 + 64, :]
                    if nev % 2 == 0:
                        nc.scalar.copy(dst, ps[:, :, :])
                    else:
                        nc.vector.tensor_copy(dst, ps[:, :, :])
                    nev += 1

            volp.__exit__(None, None, None)
            psD.__exit__(None, None, None)

            # ---------- P_W + P_H + pointwise, pipelined per d'-block ----
            # P_W ordered d'-block-outer so chunk c's P_H + pointwise can
            # trail one block behind P_W(c+1): Pool's pointwise overlaps
            # ACT/DVE evacuations, P_W matmuls fill PE between P_H chunks.
            psW = tc.tile_pool(name="psW", bufs=3, space="PSUM")
            psWp = psW.__enter__()
            for v in range(5):
                for db in range(8):
                    ps = psWp.tile([128, 4, 128], F32, tag="psW")
                    for k in range(4):
                        dp = db * 4 + k
                        nc.tensor.matmul(out=ps[:, k, :],
                                         lhsT=t1[v][:, :, dp],
                                         rhs=bw[:, :])
                    dst = t2[v][:, db * 4:db * 4 + 4, :]
                    if nev % 2 == 0:
                        nc.scalar.copy(dst, ps[:, :, :])
                    else:
                        nc.vector.tensor_copy(dst, ps[:, :, :])
                    nev += 1
            psW.__exit__(None, None, None)
            t1p.__exit__(None, None, None)

            psH = tc.tile_pool(name="psH", bufs=7, space="PSUM")
            psHp = psH.__enter__()
            psR = tc.tile_pool(name="psR", bufs=1, space="PSUM")
            psRp = psR.__enter__()
            ptwp = tc.tile_pool(name="ptw", bufs=3)
            ptw = ptwp.__enter__()

            acc_ps = psRp.tile([128, 8], F32)
            pend = {}

            def emit_chunk(c):
                rhs = [t2[v][:, c * 4:c * 4 + 4, :].rearrange(
                    "p a b -> p (a b)") for v in range(5)]
                psI = psHp.tile([128, 512], F32, tag="psH")
                psJ = psHp.tile([128, 512], F32, tag="psH")
                nc.tensor.matmul(out=psI[:, :], lhsT=bw[:, :], rhs=rhs[0])
                nc.tensor.matmul(out=psJ[:, :], lhsT=bw[:, :], rhs=rhs[1])
                psI2 = psHp.tile([128, 512], F32, tag="psH")
                psJ2 = psHp.tile([128, 512], F32, tag="psH")
                psIJ = psHp.tile([128, 512], F32, tag="psH")
                nc.tensor.matmul(out=psI2[:, :], lhsT=bw[:, :], rhs=rhs[2],
                                 start=True, stop=False)
                nc.tensor.matmul(out=psJ2[:, :], lhsT=bw[:, :], rhs=rhs[3],
                                 start=True, stop=False)
                nc.tensor.matmul(out=psIJ[:, :], lhsT=bw[:, :], rhs=rhs[4],
                                 start=True, stop=False)

                ap = ptw.tile([128, 512], BF16, tag="ap", name="ap")
                bp = ptw.tile([128, 512], BF16, tag="bp", name="bp")
                nc.scalar.mul(ap[:, :], psI[:, :], SQS)
                nc.scalar.mul(bp[:, :], psJ[:, :], SQS)

                qI = ptw.tile([128, 512], BF16, tag="qI", name="qI")
                qJ = ptw.tile([128, 512], BF16, tag="qJ", name="qJ")
                m = ptw.tile([128, 512], BF16, tag="m", name="m")
                nc.gpsimd.tensor_tensor(out=qI[:, :], in0=ap[:, :],
                                        in1=ap[:, :], op=ALU.mult)
                nc.gpsimd.tensor_tensor(out=qJ[:, :], in0=bp[:, :],
                                        in1=bp[:, :], op=ALU.mult)
                nc.vector.tensor_tensor(out=m[:, :], in0=ap[:, :],
                                        in1=bp[:, :], op=ALU.mult)
                pend[c] = (psI2, psJ2, psIJ, qI, qJ, m)

            def subs_chunk(c):
                psI2, psJ2, psIJ, qI, qJ, m = pend[c]
                nc.tensor.matmul(out=psI2[:, :], lhsT=negI[:, :], rhs=qI[:, :],
                                 start=False, stop=True)
                nc.tensor.matmul(out=psJ2[:, :], lhsT=negI[:, :], rhs=qJ[:, :],
                                 start=False, stop=True)
                nc.tensor.matmul(out=psIJ[:, :], lhsT=negI[:, :], rhs=m[:, :],
                                 start=False, stop=True)

            def finish_chunk(c):
                psI2, psJ2, psIJ, qI, qJ, m = pend.pop(c)
                ivp = ptw.tile([128, 512], BF16, tag="ivp", name="ivp")
                crp = ptw.tile([128, 512], BF16, tag="crp", name="crp")
                nc.scalar.copy(ivp[:, :], psI2[:, :])
                nc.scalar.copy(crp[:, :], psIJ[:, :])

                den = ptw.tile([128, 512], F32, tag="den", name="den")
                nc.vector.tensor_tensor(out=den[:, :], in0=psJ2[:, :],
                                        in1=ivp[:, :], op=ALU.mult)
                rec = ptw.tile([128, 512], F32, tag="rec", name="rec")
                nc.vector.reciprocal_approx_fast(out=rec[:, :], in_=den[:, :])

                t = ptw.tile([128, 512], BF16, tag="t", name="t")
                nc.vector.tensor_tensor(out=t[:, :], in0=crp[:, :],
                                        in1=rec[:, :], op=ALU.mult)
                cc = ptw.tile([128, 512], BF16, tag="cc", name="cc")
                nc.gpsimd.tensor_tensor(out=cc[:, :], in0=t[:, :],
                                        in1=crp[:, :], op=ALU.mult)
                return cc

            def reduce_chunk(c, cc):
                for k in range(4):
                    nc.tensor.matmul(out=acc_ps[:, c:c + 1],
                                     lhsT=cc[:, 128 * k:128 * k + 128],
                                     rhs=ones[:, :],
                                     start=(k == 0), stop=(k == 3))

            for c in range(8):
                emit_chunk(c)
                subs_chunk(c)
                cc = finish_chunk(c)
                reduce_chunk(c, cc)

            accs = accp.tile([128, 8], F32)
            nc.scalar.copy(accs[:, :], acc_ps[:, :])
            nc.sync.dma_start(out=out_dram[:, :], in_=accs[:, :])
            ptwp.__exit__(None, None, None)
            psR.__exit__(None, None, None)
            psH.__exit__(None, None, None)

    nc.compile()
    return nc


def kernel(y_pred: np.ndarray, y_true: np.ndarray) -> np.ndarray:
    y_pred = np.asarray(y_pred, dtype=np.float32)
    y_true = np.asarray(y_true, dtype=np.float32)

    if "nc" not in _CACHE:
        _CACHE["nc"] = _build()
    nc = _CACHE["nc"]

    ib = y_true.astype(ml_dtypes.bfloat16)
    jb = y_pred.astype(ml_dtypes.bfloat16)

    in_maps = []
    for core in range(8):
        b = core // 4
        d0 = (core % 4) * DL
        lo, hi = d0 - PAD, d0 + DL + PAD
        slo, shi = max(lo, 0), min(hi, D)
        ipk = np.zeros((104, 64, 128), ml_dtypes.bfloat16)
        jpk = np.zeros((104, 64, 128), ml_dtypes.bfloat16)
        for hb in range(2):
            hs = slice(hb * 64, hb * 64 + 64)
            p0 = 64 * hb
            ipk[p0 + slo - lo:p0 + shi - lo] = ib[b, 0, slo:shi, hs, :]
            jpk[p0 + slo - lo:p0 + shi - lo] = jb[b, 0, slo:shi, hs, :]
        in_maps.append({"i_pk": ipk, "j_pk": jpk})

    res = run_bass_kernel_spmd(nc, in_maps, core_ids=list(range(8)))
    total = 0.0
    for r in res.results:
        total += float(np.asarray(r["partials"], np.float64).sum())
    return np.float32(-total / N_TOT)


if __name__ == "__main__":
    rng = np.random.default_rng(0)
    yp = rng.standard_normal((B, 1, D, H, W), dtype=np.float32)
    yt = rng.standard_normal((B, 1, D, H, W), dtype=np.float32)
    print("loss:", kernel(yp, yt))


# revision 20
# speedup vs baseline: 1.2138x; 1.0556x over previous
"""NCC loss (9x9x9 box normalized cross-correlation) on 8 TRN2 NeuronCores.

Inputs: y_pred, y_true f32 (2,1,128,128,128). Output: scalar f32 loss.

Sharding: D axis (dim 2) split 4-ways per batch -> 8 slabs of 32 D-slices,
each with a 4-slice halo (host zero-pads volume edges). Inputs are converted
to bf16 on the host (same precision as the on-device copy the previous
version did) and packed into a [104, 64, 128] layout: partitions 0..39 hold
the 40 halo'd d-rows for h-block 0 (h 0..63), partitions 64..103 hold them
for h-block 1 (h 64..127), rows 40..63 / 104..127 are zero.

Per core, separable box filter as three matmul passes (contract D, then W,
then H) so every intermediate is a full-128-partition tile:

  prep  : I*I, J*J, I*J products in bf16                  (DVE/ACT)
  P_D   : per h, lhsT=vol[d,w] slab, rhs=banded BD[40,32] -> t1 [w,(h,d')]
  P_W   : per d', lhsT=t1[w,h], rhs=band BW[128,128]      -> t2 [h,(d',w')]
  P_H   : stationary band BH, rhs=t2 chunks of 512        -> PSUM [h',512]
  ptw   : cc = cross^2/(I_var*J_var) with the three big PSUM subtractions
          done ON THE PE via accumulating -identity matmuls, reciprocal via
          the fast bit-trick custom DVE op, final mean via ones-matmul
          reduction accumulated in PSUM.
Host: sum per-core [128,8] partials, loss = -sum / N.
"""

import math

import numpy as np
import ml_dtypes

import concourse.bacc as bacc
import concourse.tile as tile
from concourse import mybir
from concourse.bass_utils import run_bass_kernel_spmd

F32 = mybir.dt.float32
BF16 = mybir.dt.bfloat16
ALU = mybir.AluOpType
ACTF = mybir.ActivationFunctionType

B, D, H, W = 2, 128, 128, 128
DL, PAD = 32, 4
DH = DL + 2 * PAD            # 40
SQS = math.sqrt(1.0 / 729.0)
N_TOT = float(B * D * H * W)

_CACHE = {}


def _build():
    nc = bacc.Bacc(trn_type="TRN2", target_bir_lowering=False)

    i_dram = nc.dram_tensor("i_pk", [104, 64, 128], BF16, kind="ExternalInput")
    j_dram = nc.dram_tensor("j_pk", [104, 64, 128], BF16, kind="ExternalInput")
    out_dram = nc.dram_tensor("partials", [128, 8], F32, kind="ExternalOutput")

    with tile.TileContext(nc) as tc:
        with (
            tc.tile_pool(name="bands", bufs=1) as bands,
            tc.tile_pool(name="stage", bufs=1) as stage,
            tc.tile_pool(name="accp", bufs=1) as accp,
        ):
            # ---------- band / constant matrices ----------
            # BD[p, j] = 1 iff j <= p <= j+8, duplicated at partition 64.
            bd = bands.tile([104, 32], BF16)
            nc.gpsimd.memset(bd[0:40, :], 1.0)
            nc.gpsimd.affine_select(bd[0:40, :], bd[0:40, :], pattern=[[-1, 32]],
                                    compare_op=ALU.is_ge, fill=0.0,
                                    base=0, channel_multiplier=1)
            nc.gpsimd.affine_select(bd[0:40, :], bd[0:40, :], pattern=[[1, 32]],
                                    compare_op=ALU.is_ge, fill=0.0,
                                    base=8, channel_multiplier=-1)
            nc.sync.dma_start(out=bd[64:104, :], in_=bd[0:40, :])

            # BW = BH: [p, j] = 1 iff |p - j| <= 4
            bw = bands.tile([128, 128], BF16)
            nc.gpsimd.memset(bw[:, :], 1.0)
            nc.gpsimd.affine_select(bw[:, :], bw[:, :], pattern=[[-1, 128]],
                                    compare_op=ALU.is_ge, fill=0.0,
                                    base=PAD, channel_multiplier=1)
            nc.gpsimd.affine_select(bw[:, :], bw[:, :], pattern=[[1, 128]],
                                    compare_op=ALU.is_ge, fill=0.0,
                                    base=PAD, channel_multiplier=-1)

            # -identity for PE-side subtraction
            negI = bands.tile([128, 128], BF16)
            nc.gpsimd.memset(negI[:, :], -1.0)
            nc.gpsimd.affine_select(negI[:, :], negI[:, :], pattern=[[-1, 128]],
                                    compare_op=ALU.is_ge, fill=0.0,
                                    base=0, channel_multiplier=1)
            nc.gpsimd.affine_select(negI[:, :], negI[:, :], pattern=[[1, 128]],
                                    compare_op=ALU.is_ge, fill=0.0,
                                    base=0, channel_multiplier=-1)

            ones = bands.tile([128, 1], BF16)
            nc.gpsimd.memset(ones[:, :], 1.0)

            # t2 tiles live until the end
            t2 = [stage.tile([128, 32, 128], BF16, name=f"t2_{v}")
                  for v in range(5)]

            # ---------- inputs + products ----------
            # pool stack (LIFO): t1 -> psD -> vols; vols popped after P_D.
            t1p = tc.tile_pool(name="t1", bufs=1)
            t1pool = t1p.__enter__()
            t1 = [t1pool.tile([128, 128, 32], BF16, name=f"t1_{v}")
                  for v in range(5)]
            psD = tc.tile_pool(name="psD", bufs=2, space="PSUM")
            psDp = psD.__enter__()
            volp = tc.tile_pool(name="vols", bufs=1)
            vols = volp.__enter__()
            vi = vols.tile([104, 64, 128], BF16, name="vi")
            vj = vols.tile([104, 64, 128], BF16, name="vj")
            vi2 = vols.tile([104, 64, 128], BF16, name="vi2")
            vj2 = vols.tile([104, 64, 128], BF16, name="vj2")
            vij = vols.tile([104, 64, 128], BF16, name="vij")
            for q in range(4):
                s = slice(q * 16, q * 16 + 16)
                nc.sync.dma_start(out=vi[:, s, :], in_=i_dram[:, s, :])
                nc.sync.dma_start(out=vj[:, s, :], in_=j_dram[:, s, :])
            for q in range(4):
                s = slice(q * 16, q * 16 + 16)
                nc.vector.tensor_tensor(out=vi2[:, s, :], in0=vi[:, s, :],
                                        in1=vi[:, s, :], op=ALU.mult)
                nc.scalar.square(vj2[:, s, :], vj[:, s, :])
                nc.gpsimd.tensor_tensor(out=vij[:, s, :], in0=vi[:, s, :],
                                        in1=vj[:, s, :], op=ALU.mult)

            VOLS = [vi, vj, vi2, vj2, vij]

            # ---------- P_D: contract D -> t1 [w, (h 128, d' 32)] ----------
            # Pool cannot touch PSUM: evacuations alternate ACT / DVE only.
            nev = 0
            for v in range(5):
                vol = VOLS[v]
                for hb in range(2):           # 4-bank tiles of 64 h
                    ps = psDp.tile([128, 64, 32], F32, tag="psD")
                    for k in range(64):
                        h = hb * 64 + k
                        b, hl = h >> 6, h & 63
                        nc.tensor.matmul(
                            out=ps[:, k, :],
                            lhsT=vol[64 * b:64 * b + 40, hl, :],
                            rhs=bd[64 * b:64 * b + 40, :])
                    dst = t1[v][:, hb * 64:hb * 64 + 64, :]
                    if nev % 2 == 0:
                        nc.scalar.copy(dst, ps[:, :, :])
                    else:
                        nc.vector.tensor_copy(dst, ps[:, :, :])
                    nev += 1

            volp.__exit__(None, None, None)
            psD.__exit__(None, None, None)

            # ---------- P_W + P_H + pointwise, pipelined per d'-block ----
            # P_W ordered d'-block-outer so chunk c's P_H + pointwise can
            # trail one block behind P_W(c+1): Pool's pointwise overlaps
            # ACT/DVE evacuations, P_W matmuls fill PE between P_H chunks.
            psW = tc.tile_pool(name="psW", bufs=3, space="PSUM")
            psWp = psW.__enter__()
            for v in range(5):
                for db in range(4):
                    ps = psWp.tile([128, 8, 128], F32, tag="psW")
                    for k in range(8):
                        dp = db * 8 + k
                        nc.tensor.matmul(out=ps[:, k, :],
                                         lhsT=t1[v][:, :, dp],
                                         rhs=bw[:, :])
                    dst = t2[v][:, db * 8:db * 8 + 8, :]
                    if nev % 2 == 0:
                        nc.scalar.copy(dst, ps[:, :, :])
                    else:
                        nc.vector.tensor_copy(dst, ps[:, :, :])
                    nev += 1
            psW.__exit__(None, None, None)
            t1p.__exit__(None, None, None)

            psH = tc.tile_pool(name="psH", bufs=7, space="PSUM")
            psHp = psH.__enter__()
            psR = tc.tile_pool(name="psR", bufs=1, space="PSUM")
            psRp = psR.__enter__()
            ptwp = tc.tile_pool(name="ptw", bufs=3)
            ptw = ptwp.__enter__()

            acc_ps = psRp.tile([128, 8], F32)
            pend = {}

            def emit_chunk(c):
                rhs = [t2[v][:, c * 4:c * 4 + 4, :].rearrange(
                    "p a b -> p (a b)") for v in range(5)]
                psI = psHp.tile([128, 512], F32, tag="psH")
                psJ = psHp.tile([128, 512], F32, tag="psH")
                nc.tensor.matmul(out=psI[:, :], lhsT=bw[:, :], rhs=rhs[0])
                nc.tensor.matmul(out=psJ[:, :], lhsT=bw[:, :], rhs=rhs[1])
                psI2 = psHp.tile([128, 512], F32, tag="psH")
                psJ2 = psHp.tile([128, 512], F32, tag="psH")
                psIJ = psHp.tile([128, 512], F32, tag="psH")
                nc.tensor.matmul(out=psI2[:, :], lhsT=bw[:, :], rhs=rhs[2],
                                 start=True, stop=False)
                nc.tensor.matmul(out=psJ2[:, :], lhsT=bw[:, :], rhs=rhs[3],
                                 start=True, stop=False)
                nc.tensor.matmul(out=psIJ[:, :], lhsT=bw[:, :], rhs=rhs[4],
                                 start=True, stop=False)

                ap = ptw.tile([128, 512], BF16, tag="ap", name="ap")
                bp = ptw.tile([128, 512], BF16, tag="bp", name="bp")
                nc.scalar.mul(ap[:, :], psI[:, :], SQS)
                nc.scalar.mul(bp[:, :], psJ[:, :], SQS)

                qI = ptw.tile([128, 512], BF16, tag="qI", name="qI")
                qJ = ptw.tile([128, 512], BF16, tag="qJ", name="qJ")
                m = ptw.tile([128, 512], BF16, tag="m", name="m")
                nc.gpsimd.tensor_tensor(out=qI[:, :], in0=ap[:, :],
                                        in1=ap[:, :], op=ALU.mult)
                nc.gpsimd.tensor_tensor(out=qJ[:, :], in0=bp[:, :],
                                        in1=bp[:, :], op=ALU.mult)
                nc.vector.tensor_tensor(out=m[:, :], in0=ap[:, :],
                                        in1=bp[:, :], op=ALU.mult)
                pend[c] = (psI2, psJ2, psIJ, qI, qJ, m)

            def subs_chunk(c):
                psI2, psJ2, psIJ, qI, qJ, m = pend[c]
                nc.tensor.matmul(out=psI2[:, :], lhsT=negI[:, :], rhs=qI[:, :],
                                 start=False, stop=True)
                nc.tensor.matmul(out=psJ2[:, :], lhsT=negI[:, :], rhs=qJ[:, :],
                                 start=False, stop=True)
                nc.tensor.matmul(out=psIJ[:, :], lhsT=negI[:, :], rhs=m[:, :],
                                 start=False, stop=True)

            def finish_chunk(c):
                psI2, psJ2, psIJ, qI, qJ, m = pend.pop(c)
                ivp = ptw.tile([128, 512], BF16, tag="ivp", name="ivp")
                crp = ptw.tile([128, 512], BF16, tag="crp", name="crp")
                nc.scalar.copy(ivp[:, :], psI2[:, :])
                nc.scalar.copy(crp[:, :], psIJ[:, :])

                den = ptw.tile([128, 512], F32, tag="den", name="den")
                nc.vector.tensor_tensor(out=den[:, :], in0=psJ2[:, :],
                                        in1=ivp[:, :], op=ALU.mult)
                rec = ptw.tile([128, 512], F32, tag="rec", name="rec")
                nc.vector.reciprocal_approx_fast(out=rec[:, :], in_=den[:, :])

                t = ptw.tile([128, 512], BF16, tag="t", name="t")
                nc.vector.tensor_tensor(out=t[:, :], in0=crp[:, :],
                                        in1=rec[:, :], op=ALU.mult)
                cc = ptw.tile([128, 512], BF16, tag="cc", name="cc")
                nc.gpsimd.tensor_tensor(out=cc[:, :], in0=t[:, :],
                                        in1=crp[:, :], op=ALU.mult)
                return cc

            def reduce_chunk(c, cc):
                for k in range(4):
                    nc.tensor.matmul(out=acc_ps[:, c:c + 1],
                                     lhsT=cc[:, 128 * k:128 * k + 128],
                                     rhs=ones[:, :],
                                     start=(k == 0), stop=(k == 3))

            ccs = {}
            for c in range(10):
                if 1 <= c <= 8:
                    subs_chunk(c - 1)
                if c < 8:
                    emit_chunk(c)
                if 1 <= c <= 8:
                    ccs[c - 1] = finish_chunk(c - 1)
                if c >= 2:
                    reduce_chunk(c - 2, ccs.pop(c - 2))

            accs = accp.tile([128, 8], F32)
            nc.scalar.copy(accs[:, :], acc_ps[:, :])
            nc.sync.dma_start(out=out_dram[:, :], in_=accs[:, :])
            ptwp.__exit__(None, None, None)
            psR.__exit__(None, None, None)
            psH.__exit__(None, None, None)

    nc.compile()
    return nc


def kernel(y_pred: np.ndarray, y_true: np.ndarray) -> np.ndarray:
    y_pred = np.asarray(y_pred, dtype=np.float32)
    y_true = np.asarray(y_true, dtype=np.float32)

    if "nc" not in _CACHE:
        _CACHE["nc"] = _build()
    nc = _CACHE["nc"]

    ib = y_true.astype(ml_dtypes.bfloat16)
    jb = y_pred.astype(ml_dtypes.bfloat16)

    in_maps = []
    for core in range(8):
        b = core // 4
        d0 = (core % 4) * DL
        lo, hi = d0 - PAD, d0 + DL + PAD
        slo, shi = max(lo, 0), min(hi, D)
        ipk = np.zeros((104, 64, 128), ml_dtypes.bfloat16)
        jpk = np.zeros((104, 64, 128), ml_dtypes.bfloat16)
        for hb in range(2):
            hs = slice(hb * 64, hb * 64 + 64)
            p0 = 64 * hb
            ipk[p0 + slo - lo:p0 + shi - lo] = ib[b, 0, slo:shi, hs, :]
            jpk[p0 + slo - lo:p0 + shi - lo] = jb[b, 0, slo:shi, hs, :]
        in_maps.append({"i_pk": ipk, "j_pk": jpk})

    res = run_bass_kernel_spmd(nc, in_maps, core_ids=list(range(8)))
    total = 0.0
    for r in res.results:
        total += float(np.asarray(r["partials"], np.float64).sum())
    return np.float32(-total / N_TOT)


if __name__ == "__main__":
    rng = np.random.default_rng(0)
    yp = rng.standard_normal((B, 1, D, H, W), dtype=np.float32)
    yt = rng.standard_normal((B, 1, D, H, W), dtype=np.float32)
    print("loss:", kernel(yp, yt))


# revision 21
# speedup vs baseline: 1.2449x; 1.0256x over previous
"""NCC loss (9x9x9 box normalized cross-correlation) on 8 TRN2 NeuronCores.

Inputs: y_pred, y_true f32 (2,1,128,128,128). Output: scalar f32 loss.

Sharding: D axis (dim 2) split 4-ways per batch -> 8 slabs of 32 D-slices,
each with a 4-slice halo (host zero-pads volume edges). Inputs are converted
to bf16 on the host (same precision as the on-device copy the previous
version did) and packed into a [104, 64, 128] layout: partitions 0..39 hold
the 40 halo'd d-rows for h-block 0 (h 0..63), partitions 64..103 hold them
for h-block 1 (h 64..127), rows 40..63 / 104..127 are zero.

Per core, separable box filter as three matmul passes (contract D, then W,
then H) so every intermediate is a full-128-partition tile:

  prep  : I*I, J*J, I*J products in bf16                  (DVE/ACT)
  P_D   : per h, lhsT=vol[d,w] slab, rhs=banded BD[40,32] -> t1 [w,(h,d')]
  P_W   : per d', lhsT=t1[w,h], rhs=band BW[128,128]      -> t2 [h,(d',w')]
  P_H   : stationary band BH, rhs=t2 chunks of 512        -> PSUM [h',512]
  ptw   : cc = cross^2/(I_var*J_var) with the three big PSUM subtractions
          done ON THE PE via accumulating -identity matmuls, reciprocal via
          the fast bit-trick custom DVE op, final mean via ones-matmul
          reduction accumulated in PSUM.
Host: sum per-core [128,8] partials, loss = -sum / N.
"""

import math

import numpy as np
import ml_dtypes

import concourse.bacc as bacc
import concourse.tile as tile
from concourse import mybir
from concourse.bass_utils import run_bass_kernel_spmd

F32 = mybir.dt.float32
BF16 = mybir.dt.bfloat16
ALU = mybir.AluOpType
ACTF = mybir.ActivationFunctionType

B, D, H, W = 2, 128, 128, 128
DL, PAD = 32, 4
DH = DL + 2 * PAD            # 40
SQS = math.sqrt(1.0 / 729.0)
N_TOT = float(B * D * H * W)

_CACHE = {}


def _build():
    nc = bacc.Bacc(trn_type="TRN2", target_bir_lowering=False)

    i_dram = nc.dram_tensor("i_pk", [104, 64, 128], BF16, kind="ExternalInput")
    j_dram = nc.dram_tensor("j_pk", [104, 64, 128], BF16, kind="ExternalInput")
    out_dram = nc.dram_tensor("partials", [128, 8], F32, kind="ExternalOutput")

    with tile.TileContext(nc) as tc:
        with (
            tc.tile_pool(name="bands", bufs=1) as bands,
            tc.tile_pool(name="stage", bufs=1) as stage,
            tc.tile_pool(name="accp", bufs=1) as accp,
        ):
            # ---------- band / constant matrices ----------
            # BD[p, j] = 1 iff j <= p <= j+8, duplicated at partition 64.
            bd = bands.tile([104, 32], BF16)
            nc.gpsimd.memset(bd[0:40, :], 1.0)
            nc.gpsimd.affine_select(bd[0:40, :], bd[0:40, :], pattern=[[-1, 32]],
                                    compare_op=ALU.is_ge, fill=0.0,
                                    base=0, channel_multiplier=1)
            nc.gpsimd.affine_select(bd[0:40, :], bd[0:40, :], pattern=[[1, 32]],
                                    compare_op=ALU.is_ge, fill=0.0,
                                    base=8, channel_multiplier=-1)
            nc.sync.dma_start(out=bd[64:104, :], in_=bd[0:40, :])

            # BW = BH: [p, j] = 1 iff |p - j| <= 4
            bw = bands.tile([128, 128], BF16)
            nc.gpsimd.memset(bw[:, :], 1.0)
            nc.gpsimd.affine_select(bw[:, :], bw[:, :], pattern=[[-1, 128]],
                                    compare_op=ALU.is_ge, fill=0.0,
                                    base=PAD, channel_multiplier=1)
            nc.gpsimd.affine_select(bw[:, :], bw[:, :], pattern=[[1, 128]],
                                    compare_op=ALU.is_ge, fill=0.0,
                                    base=PAD, channel_multiplier=-1)

            # -identity for PE-side subtraction
            negI = bands.tile([128, 128], BF16)
            nc.gpsimd.memset(negI[:, :], -1.0)
            nc.gpsimd.affine_select(negI[:, :], negI[:, :], pattern=[[-1, 128]],
                                    compare_op=ALU.is_ge, fill=0.0,
                                    base=0, channel_multiplier=1)
            nc.gpsimd.affine_select(negI[:, :], negI[:, :], pattern=[[1, 128]],
                                    compare_op=ALU.is_ge, fill=0.0,
                                    base=0, channel_multiplier=-1)

            ones = bands.tile([128, 1], BF16)
            nc.gpsimd.memset(ones[:, :], 1.0)

            # t2 tiles live until the end
            t2 = [stage.tile([128, 32, 128], BF16, name=f"t2_{v}")
                  for v in range(5)]

            # ---------- inputs + products ----------
            # pool stack (LIFO): t1 -> psD -> vols; vols popped after P_D.
            t1p = tc.tile_pool(name="t1", bufs=1)
            t1pool = t1p.__enter__()
            t1 = [t1pool.tile([128, 128, 32], BF16, name=f"t1_{v}")
                  for v in range(5)]
            psD = tc.tile_pool(name="psD", bufs=2, space="PSUM")
            psDp = psD.__enter__()
            volp = tc.tile_pool(name="vols", bufs=1)
            vols = volp.__enter__()
            vi = vols.tile([104, 64, 128], BF16, name="vi")
            vj = vols.tile([104, 64, 128], BF16, name="vj")
            vi2 = vols.tile([104, 64, 128], BF16, name="vi2")
            vj2 = vols.tile([104, 64, 128], BF16, name="vj2")
            vij = vols.tile([104, 64, 128], BF16, name="vij")
            for q in range(8):
                s = slice(q * 8, q * 8 + 8)
                nc.sync.dma_start(out=vi[:, s, :], in_=i_dram[:, s, :])
                nc.sync.dma_start(out=vj[:, s, :], in_=j_dram[:, s, :])

            nev = 0

            def p_d(v, vol):
                nonlocal nev
                for hb in range(2):           # 4-bank tiles of 64 h
                    ps = psDp.tile([128, 64, 32], F32, tag="psD")
                    for k in range(64):
                        h = hb * 64 + k
                        b, hl = h >> 6, h & 63
                        nc.tensor.matmul(
                            out=ps[:, k, :],
                            lhsT=vol[64 * b:64 * b + 40, hl, :],
                            rhs=bd[64 * b:64 * b + 40, :])
                    dst = t1[v][:, hb * 64:hb * 64 + 64, :]
                    if nev % 2 == 0:
                        nc.scalar.copy(dst, ps[:, :, :])
                    else:
                        nc.vector.tensor_copy(dst, ps[:, :, :])
                    nev += 1

            # P_D for the raw inputs first (only DMA-gated), then products.
            p_d(0, vi)
            p_d(1, vj)
            for q in range(4):
                s = slice(q * 16, q * 16 + 16)
                nc.vector.tensor_tensor(out=vi2[:, s, :], in0=vi[:, s, :],
                                        in1=vi[:, s, :], op=ALU.mult)
                nc.scalar.square(vj2[:, s, :], vj[:, s, :])
                nc.gpsimd.tensor_tensor(out=vij[:, s, :], in0=vi[:, s, :],
                                        in1=vj[:, s, :], op=ALU.mult)
            p_d(2, vi2)
            p_d(3, vj2)
            p_d(4, vij)

            volp.__exit__(None, None, None)
            psD.__exit__(None, None, None)

            # ---------- P_W + P_H + pointwise, pipelined per d'-block ----
            # P_W ordered d'-block-outer so chunk c's P_H + pointwise can
            # trail one block behind P_W(c+1): Pool's pointwise overlaps
            # ACT/DVE evacuations, P_W matmuls fill PE between P_H chunks.
            psW = tc.tile_pool(name="psW", bufs=3, space="PSUM")
            psWp = psW.__enter__()
            for v in range(5):
                for db in range(4):
                    ps = psWp.tile([128, 8, 128], F32, tag="psW")
                    for k in range(8):
                        dp = db * 8 + k
                        nc.tensor.matmul(out=ps[:, k, :],
                                         lhsT=t1[v][:, :, dp],
                                         rhs=bw[:, :])
                    dst = t2[v][:, db * 8:db * 8 + 8, :]
                    if nev % 2 == 0:
                        nc.scalar.copy(dst, ps[:, :, :])
                    else:
                        nc.vector.tensor_copy(dst, ps[:, :, :])
                    nev += 1
            psW.__exit__(None, None, None)
            t1p.__exit__(None, None, None)

            psH = tc.tile_pool(name="psH", bufs=7, space="PSUM")
            psHp = psH.__enter__()
            psR = tc.tile_pool(name="psR", bufs=1, space="PSUM")
            psRp = psR.__enter__()
            ptwp = tc.tile_pool(name="ptw", bufs=3)
            ptw = ptwp.__enter__()

            acc_ps = psRp.tile([128, 8], F32)
            pend = {}

            def emit_chunk(c):
                rhs = [t2[v][:, c * 4:c * 4 + 4, :].rearrange(
                    "p a b -> p (a b)") for v in range(5)]
                psI = psHp.tile([128, 512], F32, tag="psH")
                psJ = psHp.tile([128, 512], F32, tag="psH")
                nc.tensor.matmul(out=psI[:, :], lhsT=bw[:, :], rhs=rhs[0])
                nc.tensor.matmul(out=psJ[:, :], lhsT=bw[:, :], rhs=rhs[1])
                psI2 = psHp.tile([128, 512], F32, tag="psH")
                psJ2 = psHp.tile([128, 512], F32, tag="psH")
                psIJ = psHp.tile([128, 512], F32, tag="psH")
                nc.tensor.matmul(out=psI2[:, :], lhsT=bw[:, :], rhs=rhs[2],
                                 start=True, stop=False)
                nc.tensor.matmul(out=psJ2[:, :], lhsT=bw[:, :], rhs=rhs[3],
                                 start=True, stop=False)
                nc.tensor.matmul(out=psIJ[:, :], lhsT=bw[:, :], rhs=rhs[4],
                                 start=True, stop=False)

                ap = ptw.tile([128, 512], BF16, tag="ap", name="ap")
                bp = ptw.tile([128, 512], BF16, tag="bp", name="bp")
                nc.scalar.mul(ap[:, :], psI[:, :], SQS)
                nc.scalar.mul(bp[:, :], psJ[:, :], SQS)

                qI = ptw.tile([128, 512], BF16, tag="qI", name="qI")
                qJ = ptw.tile([128, 512], BF16, tag="qJ", name="qJ")
                m = ptw.tile([128, 512], BF16, tag="m", name="m")
                nc.gpsimd.tensor_tensor(out=qI[:, :], in0=ap[:, :],
                                        in1=ap[:, :], op=ALU.mult)
                nc.vector.tensor_tensor(out=qJ[:, :], in0=bp[:, :],
                                        in1=bp[:, :], op=ALU.mult)
                nc.vector.tensor_tensor(out=m[:, :], in0=ap[:, :],
                                        in1=bp[:, :], op=ALU.mult)
                pend[c] = (psI2, psJ2, psIJ, qI, qJ, m)

            def subs_chunk(c):
                psI2, psJ2, psIJ, qI, qJ, m = pend[c]
                nc.tensor.matmul(out=psI2[:, :], lhsT=negI[:, :], rhs=qI[:, :],
                                 start=False, stop=True)
                nc.tensor.matmul(out=psJ2[:, :], lhsT=negI[:, :], rhs=qJ[:, :],
                                 start=False, stop=True)
                nc.tensor.matmul(out=psIJ[:, :], lhsT=negI[:, :], rhs=m[:, :],
                                 start=False, stop=True)

            def finish_chunk(c):
                psI2, psJ2, psIJ, qI, qJ, m = pend.pop(c)
                ivp = ptw.tile([128, 512], BF16, tag="ivp", name="ivp")
                crp = ptw.tile([128, 512], BF16, tag="crp", name="crp")
                nc.scalar.copy(ivp[:, :], psI2[:, :])
                nc.scalar.copy(crp[:, :], psIJ[:, :])

                den = ptw.tile([128, 512], F32, tag="den", name="den")
                nc.vector.tensor_tensor(out=den[:, :], in0=psJ2[:, :],
                                        in1=ivp[:, :], op=ALU.mult)
                rec = ptw.tile([128, 512], F32, tag="rec", name="rec")
                nc.vector.reciprocal_approx_fast(out=rec[:, :], in_=den[:, :])

                t = ptw.tile([128, 512], BF16, tag="t", name="t")
                nc.vector.tensor_tensor(out=t[:, :], in0=crp[:, :],
                                        in1=rec[:, :], op=ALU.mult)
                cc = ptw.tile([128, 512], BF16, tag="cc", name="cc")
                nc.gpsimd.tensor_tensor(out=cc[:, :], in0=t[:, :],
                                        in1=crp[:, :], op=ALU.mult)
                return cc

            def reduce_chunk(c, cc):
                for k in range(4):
                    nc.tensor.matmul(out=acc_ps[:, c:c + 1],
                                     lhsT=cc[:, 128 * k:128 * k + 128],
                                     rhs=ones[:, :],
                                     start=(k == 0), stop=(k == 3))

            ccs = {}
            for c in range(10):
                if 1 <= c <= 8:
                    subs_chunk(c - 1)
                if c < 8:
                    emit_chunk(c)
                if 1 <= c <= 8:
                    ccs[c - 1] = finish_chunk(c - 1)
                if c >= 2:
                    reduce_chunk(c - 2, ccs.pop(c - 2))

            accs = accp.tile([128, 8], F32)
            nc.scalar.copy(accs[:, :], acc_ps[:, :])
            nc.sync.dma_start(out=out_dram[:, :], in_=accs[:, :])
            ptwp.__exit__(None, None, None)
            psR.__exit__(None, None, None)
            psH.__exit__(None, None, None)

    nc.compile()
    return nc


def kernel(y_pred: np.ndarray, y_true: np.ndarray) -> np.ndarray:
    y_pred = np.asarray(y_pred, dtype=np.float32)
    y_true = np.asarray(y_true, dtype=np.float32)

    if "nc" not in _CACHE:
        _CACHE["nc"] = _build()
    nc = _CACHE["nc"]

    ib = y_true.astype(ml_dtypes.bfloat16)
    jb = y_pred.astype(ml_dtypes.bfloat16)

    in_maps = []
    for core in range(8):
        b = core // 4
        d0 = (core % 4) * DL
        lo, hi = d0 - PAD, d0 + DL + PAD
        slo, shi = max(lo, 0), min(hi, D)
        ipk = np.zeros((104, 64, 128), ml_dtypes.bfloat16)
        jpk = np.zeros((104, 64, 128), ml_dtypes.bfloat16)
        for hb in range(2):
            hs = slice(hb * 64, hb * 64 + 64)
            p0 = 64 * hb
            ipk[p0 + slo - lo:p0 + shi - lo] = ib[b, 0, slo:shi, hs, :]
            jpk[p0 + slo - lo:p0 + shi - lo] = jb[b, 0, slo:shi, hs, :]
        in_maps.append({"i_pk": ipk, "j_pk": jpk})

    res = run_bass_kernel_spmd(nc, in_maps, core_ids=list(range(8)))
    total = 0.0
    for r in res.results:
        total += float(np.asarray(r["partials"], np.float64).sum())
    return np.float32(-total / N_TOT)


if __name__ == "__main__":
    rng = np.random.default_rng(0)
    yp = rng.standard_normal((B, 1, D, H, W), dtype=np.float32)
    yt = rng.standard_normal((B, 1, D, H, W), dtype=np.float32)
    print("loss:", kernel(yp, yt))
